# revision 5
# baseline (speedup 1.0000x reference)
"""Trainium2 Bass kernel for the gnn_message_passing actor problem.

Math (reference, per persona k of P=5, p = persona[times]):
    msg  = edges @ attributes                       # [N, F]
    feat = r_k*attr + (msg*W_k)*(1-r_k)             # [N, F]
    nf   = feat / ||feat||_row                      # row L2 norm
    x    = exp((nf @ nf.T)/(T_k+eps)) * e_k
    x    = x / (max(x) + eps)
    out += tanh(x) * p[:,k][None,:] * (p[:,k][:,None] + (k==0))

Key analytic simplification: rows of nf are unit vectors so
max(nf@nf.T) == 1 (diagonal), hence max(x) = e_k*exp(1/(T_k+eps))
exactly -- no global reduction needed.  Everything is row-local:
    out_ij = sum_k tanh(exp(g_kij*s_k + b_k)) * pcol_jk * prow_ik
with s_k = 1/(T_k+eps), b_k = ln(e_k / (e_k*exp(s_k) + eps)),
prow_ik = p_ik + (k==0), pcol_jk = p_jk.

Distribution: shard N (rows) over 8 NeuronCores (512 rows each).
Each core computes msg^T for its rows ([F, 512], f-major, via
edges^T supplied pre-transposed by the host), normalizes per persona
(f-major; row sumsq via ones-matmul; 1/sqrt via exp(-0.5*ln(ss))),
AllGathers the 5 normalized feature blocks (bf16), then computes
G = nf_local @ nf_full^T per persona with TensorE, applies
exp/tanh on ScalarE and the rank-1 persona gates on VectorE,
accumulating [512, 4096] bf16 output rows.
"""

import sys

try:
    import concourse  # noqa: F401
except ImportError:  # pragma: no cover
    sys.path.insert(0, "/opt/trn_rl_repo")

import math

import ml_dtypes
import numpy as np

from concourse import bacc, tile
import concourse.mybir as mybir
from concourse.bass_utils import run_bass_kernel_spmd

N = 4096
F = 256
P = 5
NC = 8
R = N // NC  # 512 rows per core
EPS = 1e-8

BF = mybir.dt.bfloat16
F32 = mybir.dt.float32
AF = mybir.ActivationFunctionType
ALU = mybir.AluOpType

LAST_EXEC_NS = None
LAST_RESULTS = None


def _build(scale, bias, rv, wp):
    """Build + compile the per-core program. scale/bias/rv/wp: 5 floats each."""
    nc = bacc.Bacc(
        "TRN2",
        target_bir_lowering=False,
        debug=False,
        enable_asserts=True,
        num_devices=NC,
    )
    edgesT = nc.dram_tensor("edgesT", [N, R], BF, kind="ExternalInput")
    attr = nc.dram_tensor("attr", [N, F], BF, kind="ExternalInput")
    attrT = nc.dram_tensor("attrT", [F, R], F32, kind="ExternalInput")
    pcol = nc.dram_tensor("pcol", [128, P, N], BF, kind="ExternalInput")
    prow = nc.dram_tensor("prow", [128, P, 4], BF, kind="ExternalInput")
    out = nc.dram_tensor("out", [R, N], BF, kind="ExternalOutput")

    with tile.TileContext(nc) as tc:
        with tc.tile_pool(name="dram", bufs=1, space="DRAM") as dram:
            agin = dram.tile([2 * P * 128, R], BF, name="agin")
            agout = dram.tile([NC * 2 * P * 128, R], BF, name="agout",
                              addr_space="Shared")

            with tc.tile_pool(name="persist", bufs=1) as pp:
                prow_sb = pp.tile([128, P, 4], BF, name="prow_sb")
                nfT_all = pp.tile([128, 2 * P, R], BF, name="nfT_all")
                ones_col = pp.tile([128, 1], F32, name="ones_col")
                ones_row = pp.tile([1, 128], F32, name="ones_row")
                msgT = pp.tile([128, 2, R], F32, name="msgT")
                aT = pp.tile([128, 2, R], F32, name="aT")
                nc.vector.memset(ones_col[:], 1.0)
                nc.vector.memset(ones_row[:], 1.0)
                bias_sb = pp.tile([128, P], F32, name="bias_sb")
                for k in range(P):
                    nc.vector.memset(bias_sb[:, k:k + 1], float(bias[k]))
                nc.sync.dma_start(prow_sb[:], prow.ap())
                nc.sync.dma_start(
                    aT[:], attrT.ap().rearrange("(v p) i -> p v i", p=128))

                # ---------- Phase A: msgT = (edges @ attr)^T, f-major ----------
                with tc.tile_pool(name="phA", bufs=1) as pa, \
                     tc.tile_pool(name="phA_psum", bufs=1, space="PSUM") as pap:
                    A = pa.tile([128, 32, F], BF, name="A")
                    E = pa.tile([128, 32, R], BF, name="E")
                    nc.sync.dma_start(
                        A[:], attr.ap().rearrange("(t p) f -> p t f", p=128))
                    nc.sync.dma_start(
                        E[:], edgesT.ap().rearrange("(t p) i -> p t i", p=128))
                    msgT_ps = pap.tile([128, 2, R], F32, name="msgT_ps")
                    for v in range(2):
                        for t in range(32):
                            nc.tensor.matmul(
                                msgT_ps[:, v, :],
                                A[:, t, 128 * v:128 * (v + 1)],
                                E[:, t, :],
                                start=(t == 0), stop=(t == 31))
                    nc.vector.tensor_copy(msgT[:], msgT_ps[:])

                # ---------- Phase B: per-persona normalized features ----------
                with tc.tile_pool(name="phB", bufs=2) as pb, \
                     tc.tile_pool(name="phB_psum", bufs=2, space="PSUM") as pbp:
                    for k in range(P):
                        tmp = pb.tile([128, 2, R], F32, name="tmp")
                        nc.vector.tensor_scalar_mul(tmp[:], msgT[:], float(wp[k]))
                        featT = pb.tile([128, 2, R], F32, name="featT")
                        nc.vector.scalar_tensor_tensor(
                            featT[:], aT[:], float(rv[k]), tmp[:],
                            ALU.mult, ALU.add)
                        sq = pb.tile([128, 2, R], F32, name="sq")
                        nc.scalar.activation(sq[:], featT[:], AF.Square)
                        ss_ps = pbp.tile([1, R], F32, name="ss_ps")
                        for v in range(2):
                            nc.tensor.matmul(ss_ps[:], ones_col[:], sq[:, v, :],
                                             start=(v == 0), stop=(v == 1))
                        lns = pb.tile([1, R], F32, name="lns")
                        nc.scalar.activation(lns[:], ss_ps[:], AF.Ln)
                        inv = pb.tile([1, R], F32, name="inv")
                        nc.scalar.activation(inv[:], lns[:], AF.Exp, scale=-0.5)
                        invbc_ps = pbp.tile([128, R], F32, name="invbc_ps")
                        nc.tensor.matmul(invbc_ps[:], ones_row[:], inv[:],
                                         start=True, stop=True)
                        for v in range(2):
                            nc.vector.tensor_mul(
                                nfT_all[:, 2 * k + v, :], featT[:, v, :],
                                invbc_ps[:])

                # ---------- Phase C: AllGather of nfT (bf16) ----------
                nc.sync.dma_start(
                    agin.rearrange("(q p) i -> p q i", p=128), nfT_all[:])
                nc.gpsimd.collective_compute(
                    "AllGather",
                    ALU.bypass,
                    replica_groups=[list(range(NC))],
                    ins=[agin.opt()],
                    outs=[agout.opt()],
                )

                with tc.tile_pool(name="nfull", bufs=1) as pnf:
                    P_rep = pnf.tile([128, P, N], BF, name="P_rep")
                    nc.sync.dma_start(P_rep[:], pcol.ap())
                    agv = agout.rearrange(
                        "(c q p) i -> p q c i", c=NC, p=128)
                    NF = []
                    for k in range(P):
                        nf_k = pnf.tile([128, 2, NC, R], BF, name=f"NF{k}")
                        for t in range(2):
                            nc.sync.dma_start(nf_k[:, t, :, :],
                                              agv[:, 2 * k + t, :, :])
                        NF.append(nf_k)

                    # ---------- Phase D: G = nf_loc @ nf_full^T; gates ----------
                    with tc.tile_pool(name="acc_pool", bufs=2) as accp, \
                         tc.tile_pool(name="chunk", bufs=3) as chp, \
                         tc.tile_pool(name="g_psum", bufs=2, space="PSUM") as gp:
                        for m in range(4):
                            acc = accp.tile([128, N], BF, name="acc")
                            for k in range(P):
                                for h in range(2):
                                    g_ps = gp.tile([128, 2048], F32, name="g_ps")
                                    for t in range(2):
                                        for s in range(4):
                                            cblk = 4 * h + s
                                            nc.tensor.matmul(
                                                g_ps[:, 512 * s:512 * (s + 1)],
                                                nfT_all[:, 2 * k + t,
                                                        128 * m:128 * (m + 1)],
                                                NF[k][:, t, cblk, :],
                                                start=(t == 0), stop=(t == 1))
                                    x = chp.tile([128, 2048], BF, name="x")
                                    nc.scalar.activation(
                                        x[:], g_ps[:], AF.Exp,
                                        bias=bias_sb[:, k:k + 1],
                                        scale=float(scale[k]))
                                    tt = chp.tile([128, 2048], BF, name="tt")
                                    nc.scalar.activation(tt[:], x[:], AF.Tanh)
                                    seg = acc[:, 2048 * h:2048 * (h + 1)]
                                    pslice = P_rep[:, k, 2048 * h:2048 * (h + 1)]
                                    rscal = prow_sb[:, k, m:m + 1]
                                    if k == 0:
                                        nc.vector.scalar_tensor_tensor(
                                            seg, tt[:], rscal, pslice,
                                            ALU.mult, ALU.mult)
                                    else:
                                        gated = chp.tile([128, 2048], BF,
                                                         name="gated")
                                        nc.vector.scalar_tensor_tensor(
                                            gated[:], tt[:], rscal, pslice,
                                            ALU.mult, ALU.mult)
                                        nc.vector.tensor_add(seg, gated[:], seg)
                            nc.sync.dma_start(
                                out.ap()[128 * m:128 * (m + 1), :], acc[:])

    nc.compile()
    return nc


def kernel(attributes, edges, persona, T, e, r, W, times):
    global LAST_EXEC_NS, LAST_RESULTS

    attributes = np.asarray(attributes, dtype=np.float32)
    edges = np.asarray(edges, dtype=np.float32)
    persona = np.asarray(persona, dtype=np.float32)
    T = np.asarray(T, dtype=np.float64)
    e = np.asarray(e, dtype=np.float64)
    r = np.asarray(r, dtype=np.float64)
    W = np.asarray(W, dtype=np.float64)
    p = persona[int(times)]  # [N, P]

    # host-side constants (float64 precision, baked as immediates)
    s = 1.0 / (T + EPS)                      # exp scale
    mx = e * np.exp(s) + EPS                 # analytic max of x
    b = np.log(e) - np.log(mx)               # exp bias
    wp = W * (1.0 - r)                       # msg mixing weight
    rv = r.copy()                            # attr mixing weight

    nc = _build(s.tolist(), b.tolist(), rv.tolist(), wp.tolist())

    bf = ml_dtypes.bfloat16
    attr_bf = attributes.astype(bf)
    pT_bf = np.ascontiguousarray(p.T.astype(bf))          # [P, N]
    pcol_rep = np.ascontiguousarray(
        np.broadcast_to(pT_bf[None], (128, P, N)))        # [128, P, N]

    in_maps = []
    for c in range(NC):
        rows = slice(c * R, (c + 1) * R)
        edgesT_c = np.ascontiguousarray(edges[rows].T).astype(bf)   # [N, R]
        attrT_c = np.ascontiguousarray(attributes[rows].T)          # [F, R] f32
        p_loc = p[rows]                                             # [R, P]
        prow_c = p_loc.reshape(4, 128, P).transpose(1, 2, 0).copy() # [128,P,4]
        prow_c[:, 0, :] += 1.0
        in_maps.append({
            "edgesT": edgesT_c,
            "attr": attr_bf,
            "attrT": attrT_c,
            "pcol": pcol_rep,
            "prow": prow_c.astype(bf),
        })

    res = None
    try:
        res = run_bass_kernel_spmd(nc, in_maps, core_ids=list(range(NC)),
                                   trace=True)
        LAST_EXEC_NS = res.exec_time_ns
    except Exception:
        res = None
    if res is None:
        res = run_bass_kernel_spmd(nc, in_maps, core_ids=list(range(NC)))
        LAST_EXEC_NS = res.exec_time_ns
    LAST_RESULTS = res

    full = np.empty((N, N), dtype=np.float32)
    for c in range(NC):
        full[c * R:(c + 1) * R] = res.results[c]["out"].astype(np.float32)
    return full


if __name__ == "__main__":
    rng = np.random.default_rng(0)
    inputs = {
        "attributes": rng.standard_normal((N, F), dtype=np.float32),
        "edges": (rng.random((N, N)) < 0.01).astype(np.float32),
        "persona": rng.random((5, N, P), dtype=np.float32),
        "T": (rng.random(P, dtype=np.float32) * 0.5 + 0.5),
        "e": (rng.random(P, dtype=np.float32) + 0.5),
        "r": rng.random(P, dtype=np.float32),
        "W": (rng.random(P, dtype=np.float32) + 0.5),
        "times": 2,
    }
    out = kernel(**inputs)
    print("kernel ran; exec_time_ns:", LAST_EXEC_NS)
    print("out[0, :4] =", out[0, :4])


# revision 7
# speedup vs baseline: 1.1709x; 1.1709x over previous
"""Trainium2 Bass kernel for the gnn_message_passing actor problem.

Math (reference, per persona k of P=5, p = persona[times]):
    msg  = edges @ attributes                       # [N, F]
    feat = r_k*attr + (msg*W_k)*(1-r_k)             # [N, F]
    nf   = feat / ||feat||_row                      # row L2 norm
    x    = exp((nf @ nf.T)/(T_k+eps)) * e_k
    x    = x / (max(x) + eps)
    out += tanh(x) * p[:,k][None,:] * (p[:,k][:,None] + (k==0))

Key analytic simplification: rows of nf are unit vectors so
max(nf@nf.T) == 1 (diagonal), hence max(x) = e_k*exp(1/(T_k+eps))
exactly -- no global reduction needed.  Everything is row-local:
    out_ij = sum_k tanh(exp(g_kij*s_k + b_k)) * pcol_jk * prow_ik
with s_k = 1/(T_k+eps), b_k = ln(e_k / (e_k*exp(s_k) + eps)),
prow_ik = p_ik + (k==0), pcol_jk = p_jk.

Distribution: shard N (rows) over 8 NeuronCores (512 rows each).
Each core computes msg^T for its rows ([F, 512], f-major, via
edges^T supplied pre-transposed by the host), normalizes per persona
(f-major; row sumsq via ones-matmul; 1/sqrt via exp(-0.5*ln(ss))),
AllGathers each persona's normalized features (bf16) separately so
the gather pipelines under compute, then computes
G = nf_local @ nf_full^T per persona with TensorE, applies
exp/tanh on ScalarE and the rank-1 persona gates on VectorE,
accumulating [512, 4096] bf16 output rows (persona-outer so each
persona's collective hides under the previous persona's compute).
"""

import sys

try:
    import concourse  # noqa: F401
except ImportError:  # pragma: no cover
    sys.path.insert(0, "/opt/trn_rl_repo")

import ml_dtypes
import numpy as np

from concourse import bacc, tile
import concourse.mybir as mybir
from concourse.bass_utils import run_bass_kernel_spmd

N = 4096
F = 256
P = 5
NC = 8
R = N // NC  # 512 rows per core
EPS = 1e-8

BF = mybir.dt.bfloat16
F32 = mybir.dt.float32
AF = mybir.ActivationFunctionType
ALU = mybir.AluOpType

LAST_EXEC_NS = None
LAST_RESULTS = None


def _build(scale, bias, rv, wp):
    """Build + compile the per-core program. scale/bias/rv/wp: 5 floats each."""
    nc = bacc.Bacc(
        "TRN2",
        target_bir_lowering=False,
        debug=False,
        enable_asserts=True,
        num_devices=NC,
    )
    edgesT = nc.dram_tensor("edgesT", [N, R], BF, kind="ExternalInput")
    attr = nc.dram_tensor("attr", [N, F], BF, kind="ExternalInput")
    attrT = nc.dram_tensor("attrT", [F, R], F32, kind="ExternalInput")
    pcol = nc.dram_tensor("pcol", [128, P, N], BF, kind="ExternalInput")
    prow = nc.dram_tensor("prow", [128, P, 4], F32, kind="ExternalInput")
    out = nc.dram_tensor("out", [R, N], BF, kind="ExternalOutput")

    with tile.TileContext(nc) as tc:
        with tc.tile_pool(name="dram", bufs=1, space="DRAM") as dram:
            # p-major staging: per-persona AG input [128, 2, 512] (2KB/part)
            agin = [dram.tile([128 * 2, R], BF, name=f"agin{k}")
                    for k in range(P)]
            agout = [dram.tile([NC * 128 * 2, R], BF, name=f"agout{k}",
                               addr_space="Shared") for k in range(P)]

            with tc.tile_pool(name="persist", bufs=1) as pp:
                prow_sb = pp.tile([128, P, 4], F32, name="prow_sb")
                nfT_all = pp.tile([128, 2 * P, R], BF, name="nfT_all")
                ones_col = pp.tile([128, 1], F32, name="ones_col")
                ones_row = pp.tile([1, 128], F32, name="ones_row")
                msgT = pp.tile([128, 2, R], F32, name="msgT")
                aT = pp.tile([128, 2, R], F32, name="aT")
                P_rep = pp.tile([128, P, N], BF, name="P_rep")
                bias_sb = pp.tile([128, P], F32, name="bias_sb")
                nc.vector.memset(ones_col[:], 1.0)
                nc.vector.memset(ones_row[:], 1.0)
                for k in range(P):
                    nc.vector.memset(bias_sb[:, k:k + 1], float(bias[k]))
                nc.sync.dma_start(prow_sb[:], prow.ap())
                nc.sync.dma_start(P_rep[:], pcol.ap())
                nc.sync.dma_start(
                    aT[:], attrT.ap().rearrange("(v p) i -> p v i", p=128))

                # ---------- Phase A: msgT = (edges @ attr)^T, f-major ----------
                # edges streamed in 4 chunks of 8 K-tiles to overlap DMA+matmul
                with tc.tile_pool(name="phA", bufs=1) as pa, \
                     tc.tile_pool(name="phA_e", bufs=2) as pae, \
                     tc.tile_pool(name="phA_psum", bufs=1, space="PSUM") as pap:
                    A = pa.tile([128, 32, F], BF, name="A")
                    nc.sync.dma_start(
                        A[:], attr.ap().rearrange("(t p) f -> p t f", p=128))
                    eview = edgesT.ap().rearrange(
                        "(g t p) i -> g p t i", g=4, p=128)
                    msgT_ps = pap.tile([128, 2, R], F32, name="msgT_ps")
                    for g in range(4):
                        E = pae.tile([128, 8, R], BF, name="E")
                        nc.sync.dma_start(E[:], eview[g])
                        for v in range(2):
                            for t in range(8):
                                nc.tensor.matmul(
                                    msgT_ps[:, v, :],
                                    A[:, 8 * g + t, 128 * v:128 * (v + 1)],
                                    E[:, t, :],
                                    start=(g == 0 and t == 0),
                                    stop=(g == 3 and t == 7))
                    nc.vector.tensor_copy(msgT[:], msgT_ps[:])

                # ---------- Phase B: per-persona normalized features ----------
                with tc.tile_pool(name="phB", bufs=2) as pb, \
                     tc.tile_pool(name="phB_psum", bufs=2, space="PSUM") as pbp:
                    for k in range(P):
                        tmp = pb.tile([128, 2, R], F32, name="tmp")
                        nc.vector.tensor_scalar_mul(tmp[:], msgT[:], float(wp[k]))
                        featT = pb.tile([128, 2, R], F32, name="featT")
                        nc.vector.scalar_tensor_tensor(
                            featT[:], aT[:], float(rv[k]), tmp[:],
                            ALU.mult, ALU.add)
                        sq = pb.tile([128, 2, R], F32, name="sq")
                        nc.vector.tensor_mul(sq[:], featT[:], featT[:])
                        ss_ps = pbp.tile([1, R], F32, name="ss_ps")
                        for v in range(2):
                            nc.tensor.matmul(ss_ps[:], ones_col[:], sq[:, v, :],
                                             start=(v == 0), stop=(v == 1))
                        lns = pb.tile([1, R], F32, name="lns")
                        nc.scalar.activation(lns[:], ss_ps[:], AF.Ln)
                        inv = pb.tile([1, R], F32, name="inv")
                        nc.scalar.activation(inv[:], lns[:], AF.Exp, scale=-0.5)
                        invbc_ps = pbp.tile([128, R], F32, name="invbc_ps")
                        nc.tensor.matmul(invbc_ps[:], ones_row[:], inv[:],
                                         start=True, stop=True)
                        for v in range(2):
                            nc.vector.tensor_mul(
                                nfT_all[:, 2 * k + v, :], featT[:, v, :],
                                invbc_ps[:])
                        # stage + AllGather this persona immediately (p-major)
                        nc.sync.dma_start(
                            agin[k].rearrange("(p q) i -> p q i", p=128),
                            nfT_all[:, 2 * k:2 * k + 2, :])
                        nc.gpsimd.collective_compute(
                            "AllGather",
                            ALU.bypass,
                            replica_groups=[list(range(NC))],
                            ins=[agin[k].opt()],
                            outs=[agout[k].opt()],
                        )

                # ---------- Phase D: G = nf_loc @ nf_full^T; gates ----------
                with tc.tile_pool(name="accp", bufs=1) as accp, \
                     tc.tile_pool(name="nfk", bufs=2) as pnf, \
                     tc.tile_pool(name="chunk", bufs=3) as chp, \
                     tc.tile_pool(name="g_psum", bufs=2, space="PSUM") as gp:
                    accs = [accp.tile([128, N], BF, name=f"acc{m}")
                            for m in range(4)]
                    for k in range(P):
                        nf_k = pnf.tile([128, 2, NC, R], BF, name="nf_k")
                        agv = agout[k].rearrange(
                            "(c p q) i -> p q c i", c=NC, p=128)
                        for t in range(2):
                            nc.sync.dma_start(nf_k[:, t, :, :], agv[:, t, :, :])
                        for m in range(4):
                            acc = accs[m]
                            for h in range(2):
                                g_ps = gp.tile([128, 2048], F32, name="g_ps")
                                for t in range(2):
                                    for s in range(4):
                                        cblk = 4 * h + s
                                        nc.tensor.matmul(
                                            g_ps[:, 512 * s:512 * (s + 1)],
                                            nfT_all[:, 2 * k + t,
                                                    128 * m:128 * (m + 1)],
                                            nf_k[:, t, cblk, :],
                                            start=(t == 0), stop=(t == 1))
                                x = chp.tile([128, 2048], BF, name="x")
                                nc.scalar.activation(
                                    x[:], g_ps[:], AF.Exp,
                                    bias=bias_sb[:, k:k + 1],
                                    scale=float(scale[k]))
                                tt = chp.tile([128, 2048], BF, name="tt")
                                nc.scalar.activation(tt[:], x[:], AF.Tanh)
                                ts = chp.tile([128, 2048], BF, name="ts")
                                nc.vector.tensor_scalar_mul(
                                    ts[:], tt[:], prow_sb[:, k, m:m + 1])
                                seg = acc[:, 2048 * h:2048 * (h + 1)]
                                pslice = P_rep[:, k, 2048 * h:2048 * (h + 1)]
                                if k == 0:
                                    nc.vector.tensor_mul(seg, ts[:], pslice)
                                else:
                                    gated = chp.tile([128, 2048], BF,
                                                     name="gated")
                                    nc.vector.tensor_mul(gated[:], ts[:], pslice)
                                    nc.vector.tensor_add(seg, gated[:], seg)
                            if k == P - 1:
                                nc.sync.dma_start(
                                    out.ap()[128 * m:128 * (m + 1), :], acc[:])

    nc.compile()
    return nc


def kernel(attributes, edges, persona, T, e, r, W, times):
    global LAST_EXEC_NS, LAST_RESULTS

    attributes = np.asarray(attributes, dtype=np.float32)
    edges = np.asarray(edges, dtype=np.float32)
    persona = np.asarray(persona, dtype=np.float32)
    T = np.asarray(T, dtype=np.float64)
    e = np.asarray(e, dtype=np.float64)
    r = np.asarray(r, dtype=np.float64)
    W = np.asarray(W, dtype=np.float64)
    p = persona[int(times)]  # [N, P]

    # host-side constants (float64 precision, baked as immediates)
    s = 1.0 / (T + EPS)                      # exp scale
    mx = e * np.exp(s) + EPS                 # analytic max of x
    b = np.log(e) - np.log(mx)               # exp bias
    wp = W * (1.0 - r)                       # msg mixing weight
    rv = r.copy()                            # attr mixing weight

    nc = _build(s.tolist(), b.tolist(), rv.tolist(), wp.tolist())

    bf = ml_dtypes.bfloat16
    attr_bf = attributes.astype(bf)
    pT_bf = np.ascontiguousarray(p.T.astype(bf))          # [P, N]
    pcol_rep = np.ascontiguousarray(
        np.broadcast_to(pT_bf[None], (128, P, N)))        # [128, P, N]

    in_maps = []
    for c in range(NC):
        rows = slice(c * R, (c + 1) * R)
        edgesT_c = np.ascontiguousarray(edges[rows].T).astype(bf)   # [N, R]
        attrT_c = np.ascontiguousarray(attributes[rows].T)          # [F, R] f32
        p_loc = p[rows]                                             # [R, P]
        prow_c = p_loc.reshape(4, 128, P).transpose(1, 2, 0).copy() # [128,P,4]
        prow_c[:, 0, :] += 1.0
        in_maps.append({
            "edgesT": edgesT_c,
            "attr": attr_bf,
            "attrT": attrT_c,
            "pcol": pcol_rep,
            "prow": prow_c.astype(np.float32),
        })

    res = None
    try:
        res = run_bass_kernel_spmd(nc, in_maps, core_ids=list(range(NC)),
                                   trace=True)
        LAST_EXEC_NS = res.exec_time_ns
    except Exception:
        res = None
    if res is None:
        res = run_bass_kernel_spmd(nc, in_maps, core_ids=list(range(NC)))
        LAST_EXEC_NS = res.exec_time_ns
    LAST_RESULTS = res

    full = np.empty((N, N), dtype=np.float32)
    for c in range(NC):
        full[c * R:(c + 1) * R] = res.results[c]["out"].astype(np.float32)
    return full


if __name__ == "__main__":
    rng = np.random.default_rng(0)
    inputs = {
        "attributes": rng.standard_normal((N, F), dtype=np.float32),
        "edges": (rng.random((N, N)) < 0.01).astype(np.float32),
        "persona": rng.random((5, N, P), dtype=np.float32),
        "T": (rng.random(P, dtype=np.float32) * 0.5 + 0.5),
        "e": (rng.random(P, dtype=np.float32) + 0.5),
        "r": rng.random(P, dtype=np.float32),
        "W": (rng.random(P, dtype=np.float32) + 0.5),
        "times": 2,
    }
    out = kernel(**inputs)
    print("kernel ran; exec_time_ns:", LAST_EXEC_NS)
    print("out[0, :4] =", out[0, :4])
